# revision 1
# baseline (speedup 1.0000x reference)
"""BigBirdEncoder Trainium2 kernel.

Data-parallel over batch: 8 NeuronCores, core i computes sample i end-to-end
(no collectives). Per core: embedding gather (indirect DMA), 2x
(pre-RMSNorm block-sparse attention + MLP), final RMSNorm.

Layout strategy (per core):
  - Residual xT kept D-major [256, 2048] fp32, SBUF-resident.
  - RMSNorm: sum-of-squares via ones-matmul (partition reduction on PE),
    rstd = exp(-0.5*ln(msq+eps)) on ACT, broadcast over partitions via K=1
    outer-product matmul (bf16 hi/lo split for precision).
  - QKV: qT,kT D-major bf16 (lhsT=weights); v token-major bf16 with a fused
    ones column per head (v_aug) so the o-matmul also produces softmax
    denominators (33rd output row).
  - Scores computed transposed (sT = [kv, q]) per (head, 2 query blocks):
    global blocks batched, window+random as even/odd-aligned block chunks.
    exp on ACT in [128, 2048] batches, no max-subtraction (|s| < 10).
  - o-matmuls: head pairs column-packed at partition offsets 0/64 (M=33);
    denominator rows at 32/96; normalization deferred: reciprocal on DVE,
    partition-broadcast via K=2 selector matmul, one in-place multiply.
  - wo contraction uses zero-padded weight rows so garbage psum rows
    (33..63, 97..127) contribute nothing.
"""

import os
import sys

for _p in ("/opt/trn_rl_repo", "/root/.axon_site/_ro/trn_rl_repo"):
    if os.path.isdir(_p) and _p not in sys.path:
        sys.path.insert(0, _p)

import numpy as np
import ml_dtypes

import concourse.bass as bass
import concourse.mybir as mybir
import concourse.tile as tile
from concourse import bacc, bass_utils

BF16 = mybir.dt.bfloat16
F32 = mybir.dt.float32
I32 = mybir.dt.int32
AF = mybir.ActivationFunctionType
ALU = mybir.AluOpType

S = 2048
D = 256
H = 8
DH = 32
BS = 64
NB = 32
FF = 1024
L = 2
EPS = 1e-8
NT = 16          # 128-token tiles
NCORES = 8
V = 32000


def _plan_attention(rand_idx):
    """Per query block n: dedup'd window+random kv-block chunks.

    Returns plans[n] = list of (b0, nb, mult, g, po):
      b0 first block id, nb blocks (1 or 2), mult = multiplicity,
      g = column group (0..2), po = partition offset in the chunk column (0/64).
    Invariants: nb==2 chunks sit at po=0; single chunks with b0==0 sit at po=0.
    """
    plans = []
    for n in range(NB):
        blocks = sorted([
            max(n - 1, 0), n, min(n + 1, NB - 1),
            int(rand_idx[n, 0]), int(rand_idx[n, 1]),
        ])
        # dedup with multiplicity
        uniq = []
        for b in blocks:
            if uniq and uniq[-1][0] == b:
                uniq[-1][1] += 1
            else:
                uniq.append([b, 1])
        # pair adjacent blocks with equal multiplicity (any parity; odd pairs
        # are served by the 64-shifted v copy)
        chunks = []
        i = 0
        while i < len(uniq):
            if (i + 1 < len(uniq) and uniq[i + 1][0] == uniq[i][0] + 1
                    and uniq[i][1] == uniq[i + 1][1]):
                chunks.append((uniq[i][0], 2, uniq[i][1]))
                i += 2
            else:
                chunks.append((uniq[i][0], 1, uniq[i][1]))
                i += 1
        # placement: pairs take a full group; singles pack two per group
        # (lower half po=0, upper half po=64). Constraints: a b0==0 single
        # has no shifted-v copy and must sit at po=0; a b0==NB-1 odd single
        # must sit at po=64.
        placement = []
        g = 0
        lowers, uppers = [], []
        pairs = [c for c in chunks if c[1] == 2]
        singles = [c for c in chunks if c[1] == 1]
        singles.sort(key=lambda c: 0 if c[0] == 0 else (1 if c[0] == NB - 1 else 2))
        for (b0, nb, m) in pairs:
            placement.append((b0, 2, m, g, 0))
            g += 1
        for (b0, nb, m) in singles:
            if b0 == 0:
                want = 0
            elif b0 == NB - 1 and b0 % 2 == 1:
                want = 64
            else:
                want = None
            if want == 0 or (want is None and not uppers and lowers):
                gg = lowers.pop(0) if lowers else None
                if gg is None:
                    gg = g
                    g += 1
                    uppers.append(gg)
                placement.append((b0, 1, m, gg, 0))
            else:
                gg = uppers.pop(0) if uppers else None
                if gg is None:
                    gg = g
                    g += 1
                    lowers.append(gg)
                    if want == 64:
                        placement.append((b0, 1, m, gg, 64))
                        lowers.pop()
                        lowers.append(gg)
                        continue
                placement.append((b0, 1, m, gg, 64))
        assert g <= 3, (n, chunks, placement)
        used = set()
        for (b0, nb, m, gg, po) in placement:
            used.add((gg, po))
            if nb == 2:
                used.add((gg, 64))
        free = [(gg, po) for gg in range(3) for po in (0, 64)
                if (gg, po) not in used]
        plans.append((placement, free))
    return plans


def build_kernel(rand_idx):
    phase = os.environ.get("K_PHASE", "full")
    plans = _plan_attention(rand_idx)
    nc = bacc.Bacc("TRN2", target_bir_lowering=False, debug=False,
                   num_devices=NCORES, detect_race_conditions=False)

    ids_d = nc.dram_tensor("ids", [128, NT], I32, kind="ExternalInput").ap()
    emb_d = nc.dram_tensor("emb", [V, D], F32, kind="ExternalInput").ap()
    wq_d = nc.dram_tensor("wq", [L, 2, 128, D], BF16, kind="ExternalInput").ap()
    wk_d = nc.dram_tensor("wk", [L, 2, 128, D], BF16, kind="ExternalInput").ap()
    wv_d = nc.dram_tensor("wv", [L, 2, 128, D], BF16, kind="ExternalInput").ap()
    wop_d = nc.dram_tensor("wop", [L, 4, 128, D], BF16, kind="ExternalInput").ap()
    w1_d = nc.dram_tensor("w1", [L, 2, 128, FF], BF16, kind="ExternalInput").ap()
    w2_d = nc.dram_tensor("w2", [L, 8, 128, D], BF16, kind="ExternalInput").ap()
    onesr_d = nc.dram_tensor("onesr", [1, 128], BF16, kind="ExternalInput").ap()
    onesc_d = nc.dram_tensor("onesc", [128, 1], BF16, kind="ExternalInput").ap()
    ident_d = nc.dram_tensor("ident", [128, 128], F32, kind="ExternalInput").ap()
    sel2_d = nc.dram_tensor("sel2", [2, 128], BF16, kind="ExternalInput").ap()
    fln_d = nc.dram_tensor("fln", [128, 2], F32, kind="ExternalInput").ap()
    out_d = nc.dram_tensor("out", [S, D], F32, kind="ExternalOutput").ap()

    from contextlib import ExitStack
    with tile.TileContext(nc) as tc, ExitStack() as ctx:
        ep = ctx.enter_context
        consts = ep(tc.tile_pool(name="consts", bufs=1))
        wpool = ep(tc.tile_pool(name="wpool", bufs=1))
        xpool = ep(tc.tile_pool(name="xpool", bufs=1))
        expp = ep(tc.tile_pool(name="expp", bufs=4))
        rp = ep(tc.tile_pool(name="rp", bufs=1))
        stage = ep(tc.tile_pool(name="stage", bufs=2))
        small = ep(tc.tile_pool(name="small", bufs=1))
        psum = ep(tc.tile_pool(name="psum", bufs=1, space="PSUM"))
        if True:
            # ---- constants ----
            ids_t = consts.tile([128, NT], I32)
            nc.sync.dma_start(out=ids_t, in_=ids_d)
            onesr = consts.tile([1, 128], BF16)
            nc.sync.dma_start(out=onesr, in_=onesr_d)
            onesc = consts.tile([128, 1], BF16)
            nc.sync.dma_start(out=onesc, in_=onesc_d)
            ident = consts.tile([128, 128], F32)
            nc.sync.dma_start(out=ident, in_=ident_d)
            sel2a = consts.tile([1, 128], BF16)
            nc.sync.dma_start(out=sel2a, in_=sel2_d[0:1, :])
            sel2b = consts.tile([1, 128], BF16)
            nc.sync.dma_start(out=sel2b, in_=sel2_d[1:2, :])
            fln_t = consts.tile([128, 2], F32)
            nc.sync.dma_start(out=fln_t, in_=fln_d)
            eps_t = consts.tile([1, 1], F32)
            nc.vector.memset(eps_t, EPS)

            # ---- persistent activations ----
            xT = xpool.tile([128, 2, S], F32, tag="xT")
            qTt = xpool.tile([128, 2, S], BF16, tag="qT")
            kTt = xpool.tile([128, 2, S], BF16, tag="kT")
            oT = xpool.tile([128, 4, S], BF16, tag="oT")
            nc.vector.memset(oT, 0.0)
            VW = H * 33 + 32          # 296: 8x(32 v + 1 ones) + zero tail
            vA = xpool.tile([128, NT, VW], BF16, tag="vA")
            vS = xpool.tile([128, NT - 1, VW], BF16, tag="vS")
            # ones columns + zero tail persist across layers (evacs only
            # write the 32 v columns of each head)
            vA4 = vA[:, :, 0:H * 33].rearrange("p a (h c) -> p a h c", c=33)
            nc.vector.memset(vA4[:, :, :, 32:33], 1.0)
            nc.vector.memset(vA[:, :, H * 33:VW], 0.0)

            # ---- embedding gather + transpose to D-major ----
            for t in range(NT):
                xtok = stage.tile([128, D], F32, tag="xtok")
                nc.gpsimd.indirect_dma_start(
                    out=xtok, out_offset=None, in_=emb_d,
                    in_offset=bass.IndirectOffsetOnAxis(ap=ids_t[:, t:t + 1], axis=0),
                )
                for c in range(2):
                    tp = psum.tile([128, 512], F32, tag="b1", bufs=4)
                    nc.tensor.transpose(tp[:, 0:128],
                                        xtok[:, c * 128:(c + 1) * 128], ident)
                    nc.vector.tensor_copy(xT[:, c, t * 128:(t + 1) * 128],
                                          tp[:, 0:128])

            def rmsnorm_factors():
                """-> (rh, rl) bf16 hi/lo split of per-token rstd [1, S]."""
                sq = xpool.tile([128, 2, S], BF16, tag="xn")
                for po in range(2):
                    nc.vector.tensor_tensor(sq[:, po, :], xT[:, po, :],
                                            xT[:, po, :], op=ALU.mult)
                ssq = psum.tile([1, S], F32, tag="big4", bufs=1)
                for nt in range(4):
                    sl = slice(nt * 512, (nt + 1) * 512)
                    for c in range(2):
                        nc.tensor.matmul(ssq[:, sl], onesc, sq[:, c, sl],
                                         start=(c == 0), stop=(c == 1))
                rstd = small.tile([1, S], F32, tag="rstd")
                nc.scalar.activation(rstd, ssq, AF.Ln, bias=eps_t[:, :],
                                     scale=1.0 / D)
                nc.scalar.activation(rstd, rstd, AF.Exp, scale=-0.5)
                rh = small.tile([1, S], BF16, tag="rh")
                nc.vector.tensor_copy(rh, rstd)
                rl = small.tile([1, S], BF16, tag="rl")
                nc.vector.tensor_tensor(rl, rstd, rh, op=ALU.subtract)
                return rh, rl

            def bcast_rstd(rh, rl):
                """[128, S] psum broadcast of rstd over partitions."""
                bc = psum.tile([128, S], F32, tag="big4", bufs=1)
                for nt in range(4):
                    sl = slice(nt * 512, (nt + 1) * 512)
                    nc.tensor.matmul(bc[:, sl], onesr, rh[:, sl],
                                     start=True, stop=False)
                    nc.tensor.matmul(bc[:, sl], onesr, rl[:, sl],
                                     start=False, stop=True)
                return bc

            def make_xn():
                rh, rl = rmsnorm_factors()
                bc = bcast_rstd(rh, rl)
                xn = xpool.tile([128, 2, S], BF16, tag="xn")
                for po in range(2):
                    nc.vector.tensor_tensor(xn[:, po, :], xT[:, po, :], bc,
                                            op=ALU.mult)
                return xn

            n_layers = 0 if phase == "embed" else (1 if phase != "full" else L)
            for l in range(n_layers):
                # ---- layer weights ----
                wq_t = wpool.tile([128, 2, D], BF16, tag="wq")
                wk_t = wpool.tile([128, 2, D], BF16, tag="wk")
                wv_t = wpool.tile([128, 2, D], BF16, tag="wv")
                for c in range(2):
                    nc.sync.dma_start(out=wq_t[:, c, :], in_=wq_d[l, c])
                    nc.sync.dma_start(out=wk_t[:, c, :], in_=wk_d[l, c])
                    nc.sync.dma_start(out=wv_t[:, c, :], in_=wv_d[l, c])
                wop_t = wpool.tile([128, 4, D], BF16, tag="wop")
                for hp in range(4):
                    nc.sync.dma_start(out=wop_t[:, hp, :], in_=wop_d[l, hp])
                w1_t = wpool.tile([128, 2, FF], BF16, tag="w1")
                for c in range(2):
                    nc.sync.dma_start(out=w1_t[:, c, :], in_=w1_d[l, c])
                w2_t = wpool.tile([128, 8, D], BF16, tag="w2")
                for kc in range(8):
                    nc.sync.dma_start(out=w2_t[:, kc, :], in_=w2_d[l, kc])

                # ---- norm 1 + QKV ----
                xn = make_xn()
                for (wt, dstT) in ((wq_t, qTt), (wk_t, kTt)):
                    for po in range(2):
                        pp = psum.tile([128, S], F32, tag="big4", bufs=1)
                        for nt in range(4):
                            sl = slice(nt * 512, (nt + 1) * 512)
                            for c in range(2):
                                nc.tensor.matmul(
                                    pp[:, sl],
                                    wt[:, c, po * 128:(po + 1) * 128],
                                    xn[:, c, sl],
                                    start=(c == 0), stop=(c == 1))
                        nc.vector.tensor_copy(dstT[:, po, :], pp)
                for sg in range(8):
                    vp = psum.tile([128, 512], F32, tag="b1", bufs=4)
                    for stl in range(2):
                        st = sg * 2 + stl
                        for c in range(2):
                            nc.tensor.matmul(
                                vp[:, stl * 256:(stl + 1) * 256],
                                xn[:, c, st * 128:(st + 1) * 128],
                                wv_t[:, c, :],
                                start=(c == 0), stop=(c == 1))
                    nc.vector.tensor_copy(
                        vA[:, sg * 2:(sg + 1) * 2, 0:H * 33]
                        .rearrange("p a (h c) -> p a h c", c=33)[:, :, :, 0:32],
                        vp.rearrange("p (a h c) -> p a h c", a=2, c=32))
                # shifted-by-64 copy of v_aug (serves odd-aligned chunks)
                nc.gpsimd.dma_start(out=vS[0:64, :, :],
                                    in_=vA[64:128, 0:NT - 1, :])
                nc.gpsimd.dma_start(out=vS[64:128, :, :],
                                    in_=vA[0:64, 1:NT, :])

                def v_slice(b0, nb, po, h):
                    """lhsT [64*nb, 64] for kv tokens [64*b0, 64*(b0+nb))
                    readable at partition offset po. 64-wide so the o-matmul
                    writes full partition halves (cols 33+ hit other heads'
                    data / the zero tail; those rows are killed by wo_pad)."""
                    hs = slice(h * 33, h * 33 + 64)
                    if nb == 2:
                        if b0 % 2 == 0:
                            return vA[:, b0 // 2, hs]
                        return vS[:, (b0 - 1) // 2, hs]
                    if po == 64 * (b0 % 2):
                        return vA[po:po + 64, b0 // 2, hs]
                    if b0 % 2 == 1:      # odd block at po=0 via shifted copy
                        return vS[0:64, (b0 - 1) // 2, hs]
                    # even block at po=64 via shifted copy (b0 >= 2 guaranteed)
                    return vS[64:128, b0 // 2 - 1, hs]

                if phase == "qkv":
                    break
                sub = phase[4:] if phase.startswith("attn") else ""
                # ---- block-sparse attention ----
                for hg in range(2):
                    for qt in range(4):
                        etiles = []
                        for g2 in range(4):
                            j2 = qt * 4 + g2
                            sc = psum.tile([128, 4, 512], F32, tag="big4", bufs=1)
                            for hh in range(4):
                                pb = 32 * hh
                                nc.tensor.matmul(
                                    sc[0:128, hh, 0:128],
                                    kTt[pb:pb + 32, hg, 0:128],
                                    qTt[pb:pb + 32, hg, j2 * 128:(j2 + 1) * 128],
                                    tile_position=(pb, 0),
                                    start=True, stop=True)
                            ksc = os.environ.get("K_SC", "gwf")
                            for ln_ in range(2 * ("w" in ksc)):
                                n = 2 * j2 + ln_
                                for (b0, nb, m, g, po) in plans[n][0]:
                                    co = 128 + ln_ * 192 + g * 64
                                    for hh in range(4):
                                        pb = 32 * hh
                                        nc.tensor.matmul(
                                            sc[po:po + 64 * nb, hh, co:co + 64],
                                            kTt[pb:pb + 32, hg,
                                                b0 * 64:(b0 + nb) * 64],
                                            qTt[pb:pb + 32, hg,
                                                n * 64:(n + 1) * 64],
                                            tile_position=(pb, po),
                                            start=True, stop=True)
                                for (g, po) in (plans[n][1] if "f" in ksc else []):
                                    # fill unused score slots so exp reads
                                    # only written psum (dummy K=32 matmul,
                                    # same shape as a real chunk matmul)
                                    co = 128 + ln_ * 192 + g * 64
                                    for hh in range(4):
                                        pb = 32 * hh
                                        nc.tensor.matmul(
                                            sc[po:po + 64, hh, co:co + 64],
                                            kTt[pb:pb + 32, hg, 0:64],
                                            qTt[pb:pb + 32, hg,
                                                n * 64:(n + 1) * 64],
                                            tile_position=(pb, po),
                                            start=True, stop=True)
                            et = expp.tile([128, 4, 512], BF16, tag="expt")
                            if os.environ.get("K_SC", "gwf") == "g":
                                nc.scalar.activation(et[:, :, 0:128],
                                                     sc[:, :, 0:128], AF.Exp)
                            else:
                                nc.scalar.activation(et, sc, AF.Exp)
                            etiles.append(et)
                        for ph in range(2 * (sub != "1")):
                            hp = hg * 2 + ph
                            op_ = psum.tile([128, 512], F32, tag="b1", bufs=4)
                            mms = []   # (out, lhsT, rhs, tpos, region=hl)
                            for hl in range(2):
                                for g2 in range(4):
                                    hh = ph * 2 + hl
                                    h = hg * 4 + hh
                                    mms.append(((
                                        op_[hl * 64:hl * 64 + 64,
                                            g2 * 128:(g2 + 1) * 128],
                                        vA[:, 0, h * 33:h * 33 + 64],
                                        etiles[g2][:, hh, 0:128],
                                        (0, hl * 64)), hl))
                            for hl in range(2):
                                # row-0 chunks (pairs + po=0 singles) first,
                                # then a full-height zero-spacer, then row-64
                                # singles: row-disjoint K=64 matmuls touching
                                # the same psum region must not overlap in
                                # the PE array.
                                row0, row64 = [], []
                                for g2 in range(4):
                                    for ln_ in range(2):
                                        n = 2 * (qt * 4 + g2) + ln_
                                        for (b0, nb, m, g, po) in plans[n][0]:
                                            co = 128 + ln_ * 192 + g * 64
                                            hh = ph * 2 + hl
                                            h = hg * 4 + hh
                                            vsl = v_slice(b0, nb, po, h)
                                            ent = ((
                                                op_[hl * 64:hl * 64 + 64,
                                                    (g2 * 2 + ln_) * 64:
                                                    (g2 * 2 + ln_ + 1) * 64],
                                                vsl,
                                                etiles[g2][po:po + 64 * nb,
                                                           hh, co:co + 64],
                                                (po, hl * 64)), hl)
                                            dst = row64 if (nb == 1 and po == 64) else row0
                                            for _ in range(m):
                                                dst.append(ent)
                                mms.extend(row0)
                                if row64:
                                    mms.append(((
                                        op_[hl * 64:hl * 64 + 32, 0:64],
                                        vA[:, 0, H * 33:H * 33 + 32],
                                        etiles[0][:, ph * 2 + hl, 0:64],
                                        (0, hl * 64)), hl))
                                    mms.extend(row64)
                            mms.sort(key=lambda e: e[1])  # hl-major order
                            last_of = {}
                            seen = set()
                            for i, (_, reg) in enumerate(mms):
                                last_of[reg] = i
                            for i, ((o_ap, l_ap, r_ap, tpos), reg) in enumerate(mms):
                                st = reg not in seen
                                seen.add(reg)
                                nc.tensor.matmul(
                                    o_ap, l_ap, r_ap, tile_position=tpos,
                                    start=st, stop=(last_of[reg] == i))
                            qsl = slice(qt * 512, (qt + 1) * 512)
                            nc.vector.tensor_copy(oT[:, hp, qsl], op_)

                if sub in ("1", "2"):
                    continue
                # ---- softmax normalization ----
                # Denominators sit (bf16) at oT rows 32/96. DMA-pack them to
                # [16, 8, 128], one exact reciprocal, DMA-unpack to row form.
                den16 = rp.tile([16, 8, 128], BF16, tag="den")
                for hp in range(4):
                    for r in range(2):
                        nc.gpsimd.dma_start(
                            out=den16[:, 2 * hp + r, :],
                            in_=oT[32 + 64 * r:33 + 64 * r, hp, :])
                den16R = rp.tile([16, 8, 128], BF16, tag="denR")
                with nc.allow_low_precision("softmax denom recip in bf16"):
                    nc.vector.reciprocal(den16R, den16)
                if sub == "3":
                    continue
                R_all = rp.tile([1, 8, S], BF16, tag="Rall")
                for hp in range(4):
                    for r in range(2):
                        nc.gpsimd.dma_start(
                            out=R_all[:, 2 * hp + r, :],
                            in_=den16R[:, 2 * hp + r, :])
                for hp in range(4):
                    bc2 = psum.tile([128, S], F32, tag="big4", bufs=1)
                    for nt in range(4):
                        sl = slice(nt * 512, (nt + 1) * 512)
                        nc.tensor.matmul(bc2[:, sl], sel2a,
                                         R_all[:, 2 * hp, sl],
                                         start=True, stop=False)
                        nc.tensor.matmul(bc2[:, sl], sel2b,
                                         R_all[:, 2 * hp + 1, sl],
                                         start=False, stop=True)
                    nc.vector.tensor_tensor(oT[:, hp, :], oT[:, hp, :],
                                            bc2, op=ALU.mult)

                if sub in ("3", "4"):
                    continue
                # ---- wo + residual ----
                for po in range(2):
                    for nt in range(4):
                        sl = slice(nt * 512, (nt + 1) * 512)
                        wp = psum.tile([128, 512], F32, tag="b1", bufs=4)
                        for hp in range(4):
                            nc.tensor.matmul(
                                wp, wop_t[:, hp, po * 128:(po + 1) * 128],
                                oT[:, hp, sl],
                                start=(hp == 0), stop=(hp == 3))
                        nc.vector.tensor_tensor(xT[:, po, sl], xT[:, po, sl],
                                                wp, op=ALU.add)

                if phase.startswith("attn"):
                    break
                # ---- norm 2 + FFN ----
                xn2 = make_xn()
                for nt in range(4):
                    sl = slice(nt * 512, (nt + 1) * 512)
                    fg = stage.tile([128, 8, 512], BF16, tag="f1g")
                    for po8 in range(8):
                        fp_ = psum.tile([128, 512], F32, tag="b1", bufs=4)
                        for c in range(2):
                            nc.tensor.matmul(
                                fp_, w1_t[:, c, po8 * 128:(po8 + 1) * 128],
                                xn2[:, c, sl],
                                start=(c == 0), stop=(c == 1))
                        nc.scalar.activation(fg[:, po8, :], fp_,
                                             AF.Gelu_apprx_tanh)
                    for po in range(2):
                        f2p = psum.tile([128, 512], F32, tag="b1", bufs=4)
                        for kc in range(8):
                            nc.tensor.matmul(
                                f2p, w2_t[:, kc, po * 128:(po + 1) * 128],
                                fg[:, kc, :],
                                start=(kc == 0), stop=(kc == 7))
                        nc.vector.tensor_tensor(xT[:, po, sl], xT[:, po, sl],
                                                f2p, op=ALU.add)

            # ---- final RMSNorm (with final_ln_w) + transpose out ----
            if phase == "full":
                rh, rl = rmsnorm_factors()
                bcf = bcast_rstd(rh, rl)
            xnF = xpool.tile([128, 2, S], F32, tag="qT")
            for po in range(2):
                if phase == "full":
                    nc.vector.tensor_tensor(xnF[:, po, :], xT[:, po, :], bcf,
                                            op=ALU.mult)
                    nc.vector.tensor_scalar(
                        out=xnF[:, po, :], in0=xnF[:, po, :],
                        scalar1=fln_t[:, po:po + 1], scalar2=None, op0=ALU.mult)
                else:
                    nc.vector.tensor_copy(xnF[:, po, :], xT[:, po, :])
            for t in range(NT):
                osb = stage.tile([128, D], F32, tag="osb")
                for po in range(2):
                    tp = psum.tile([128, 512], F32, tag="b1", bufs=4)
                    nc.tensor.transpose(
                        tp[:, 0:128], xnF[:, po, t * 128:(t + 1) * 128], ident)
                    nc.vector.tensor_copy(osb[:, po * 128:(po + 1) * 128],
                                          tp[:, 0:128])
                nc.sync.dma_start(out=out_d[t * 128:(t + 1) * 128, :], in_=osb)

    nc.compile()
    return nc


def prep_in_maps(inputs):
    bf = ml_dtypes.bfloat16
    ids = np.asarray(inputs["input_ids"]).astype(np.int32)          # [8, S]
    rand_idx = np.asarray(inputs["rand_idx"]).astype(np.int32)      # [NB, 2]
    emb = np.ascontiguousarray(np.asarray(inputs["emb"], np.float32))
    ln1 = np.asarray(inputs["ln1_w"], np.float32)
    ln2 = np.asarray(inputs["ln2_w"], np.float32)
    wq = np.asarray(inputs["wq"], np.float32)
    wk = np.asarray(inputs["wk"], np.float32)
    wv = np.asarray(inputs["wv"], np.float32)
    wo = np.asarray(inputs["wo"], np.float32)
    w1 = np.asarray(inputs["w1"], np.float32)
    w2 = np.asarray(inputs["w2"], np.float32)
    fln = np.asarray(inputs["final_ln_w"], np.float32)

    scale = 1.0 / np.sqrt(DH)
    wq_p = np.ascontiguousarray(
        (wq * ln1[:, :, None] * scale).reshape(L, 2, 128, D)).astype(bf)
    wk_p = np.ascontiguousarray(
        (wk * ln1[:, :, None]).reshape(L, 2, 128, D)).astype(bf)
    wv_p = np.ascontiguousarray(
        (wv * ln1[:, :, None]).reshape(L, 2, 128, D)).astype(bf)
    wop = np.zeros((L, 4, 128, D), np.float32)
    for hp in range(4):
        wop[:, hp, 0:32, :] = wo[:, 64 * hp:64 * hp + 32, :]
        wop[:, hp, 64:96, :] = wo[:, 64 * hp + 32:64 * hp + 64, :]
    wop = wop.astype(bf)
    w1_p = np.ascontiguousarray(
        (w1 * ln2[:, :, None]).reshape(L, 2, 128, FF)).astype(bf)
    w2_p = np.ascontiguousarray(w2.reshape(L, 8, 128, D)).astype(bf)

    sel2 = np.zeros((2, 128), bf)
    sel2[0, :64] = 1.0
    sel2[1, 64:] = 1.0
    common = {
        "emb": emb,
        "wq": wq_p, "wk": wk_p, "wv": wv_p, "wop": wop,
        "w1": w1_p, "w2": w2_p,
        "onesr": np.ones((1, 128), bf),
        "onesc": np.ones((128, 1), bf),
        "ident": np.eye(128, dtype=np.float32),
        "sel2": sel2,
        "fln": np.ascontiguousarray(fln.reshape(2, 128).T),
    }
    in_maps = []
    for c in range(NCORES):
        m = dict(common)
        m["ids"] = np.ascontiguousarray(ids[c].reshape(NT, 128).T)
        in_maps.append(m)
    return in_maps, rand_idx


_NC_CACHE = {}


def get_nc(rand_idx):
    key = (os.environ.get("K_PHASE", "full"), os.environ.get("K_SC", "gwf"),
           os.environ.get("K_O", "gw"), os.environ.get("K_RECIP", ""),
           rand_idx.tobytes())
    if key not in _NC_CACHE:
        _NC_CACHE[key] = build_kernel(rand_idx)
    return _NC_CACHE[key]


def kernel(**inputs):
    in_maps, rand_idx = prep_in_maps(inputs)
    nc = get_nc(rand_idx)
    res = bass_utils.run_bass_kernel_spmd(nc, in_maps, list(range(NCORES)),
                                          trace=False)
    out = np.stack([np.asarray(res.results[c]["out"], np.float32)
                    for c in range(NCORES)])
    return out



# revision 17
# speedup vs baseline: 1.0346x; 1.0346x over previous
"""BigBirdEncoder Trainium2 kernel.

Data-parallel over batch: 8 NeuronCores, core i computes sample i end-to-end
(no collectives). Per core: embedding gather (indirect DMA), 2x
(pre-RMSNorm block-sparse attention + MLP), final RMSNorm.

Layout strategy (per core):
  - Residual xT kept D-major [256, 2048] fp32, SBUF-resident.
  - RMSNorm: sum-of-squares via ones-matmul (partition reduction on PE),
    rstd = exp(-0.5*ln(msq+eps)) on ACT, broadcast over partitions via K=1
    outer-product matmul (bf16 hi/lo split for precision).
  - QKV: qT,kT D-major bf16 (lhsT=weights); v token-major bf16 with a fused
    ones column per head (v_aug) so the o-matmul also produces softmax
    denominators (33rd output row).
  - Scores computed transposed (sT = [kv, q]) per head quadrant, minimizing
    PE instruction count (the PE sequencer at ~97ns/matmul is the kernel's
    critical path):
      * window: one dense [128 kv, <=256 q] matmul per even kv-block pair
        p=(2p,2p+1), serving q blocks 2p-1..2p+2; the two "corner"
        (q,kv-half) combos that fall outside the window are computed but
        never read by the o-matmuls (their K-ranges exclude those rows).
      * global: [128 kv, 256 q] matmuls (blocks 0,1 for all q).
      * random: per-n [128 kv, 64 q] with both rand blocks pre-gathered
        into kRand (column gather) / vRand (partition gather) via DMA.
    Window clipping multiplicity at n=0/31 handled by x2 scaling of the
    exp tile region (exactly matches duplicated-slot softmax counting).
  - o-matmuls per (head, q-block): 1 global (N=512 per qt), 1 window middle
    (K=128), lower/upper half-pair edges (K=64), 1 random (K=128 via vRand);
    denominator rows at 32/96 via the v ones-column; normalization deferred:
    reciprocal on DVE, partition-broadcast via K=2 selector matmul, one
    in-place multiply.
  - wo contraction uses zero-padded weight rows so garbage psum rows
    (33..63, 97..127) contribute nothing.
"""

import os
import sys

for _p in ("/opt/trn_rl_repo", "/root/.axon_site/_ro/trn_rl_repo"):
    if os.path.isdir(_p) and _p not in sys.path:
        sys.path.insert(0, _p)

import numpy as np
import ml_dtypes

import concourse.bass as bass
import concourse.mybir as mybir
import concourse.tile as tile
from concourse import bacc, bass_utils

BF16 = mybir.dt.bfloat16
F32 = mybir.dt.float32
I32 = mybir.dt.int32
AF = mybir.ActivationFunctionType
ALU = mybir.AluOpType

S = 2048
D = 256
H = 8
DH = 32
BS = 64
NB = 32
FF = 1024
L = 2
EPS = 1e-8
NT = 16          # 128-token tiles
NCORES = 8
V = 32000
NP = NB // 2     # 16 even-aligned kv block pairs


def build_kernel(rand_idx):
    phase = os.environ.get("K_PHASE", "full")
    nc = bacc.Bacc("TRN2", target_bir_lowering=False, debug=False,
                   num_devices=NCORES, detect_race_conditions=False)

    ids_d = nc.dram_tensor("ids", [128, NT], I32, kind="ExternalInput").ap()
    emb_d = nc.dram_tensor("emb", [V, D], F32, kind="ExternalInput").ap()
    wq_d = nc.dram_tensor("wq", [L, 2, 128, D], BF16, kind="ExternalInput").ap()
    wk_d = nc.dram_tensor("wk", [L, 2, 128, D], BF16, kind="ExternalInput").ap()
    wv_d = nc.dram_tensor("wv", [L, 2, 128, D], BF16, kind="ExternalInput").ap()
    wop_d = nc.dram_tensor("wop", [L, 4, 128, D], BF16, kind="ExternalInput").ap()
    w1_d = nc.dram_tensor("w1", [L, 2, 128, FF], BF16, kind="ExternalInput").ap()
    w2_d = nc.dram_tensor("w2", [L, 8, 128, D], BF16, kind="ExternalInput").ap()
    onesr_d = nc.dram_tensor("onesr", [1, 128], BF16, kind="ExternalInput").ap()
    onesc_d = nc.dram_tensor("onesc", [128, 1], BF16, kind="ExternalInput").ap()
    ident_d = nc.dram_tensor("ident", [128, 128], F32, kind="ExternalInput").ap()
    sel2_d = nc.dram_tensor("sel2", [2, 128], BF16, kind="ExternalInput").ap()
    fln_d = nc.dram_tensor("fln", [128, 2], F32, kind="ExternalInput").ap()
    out_d = nc.dram_tensor("out", [S, D], F32, kind="ExternalOutput").ap()

    from contextlib import ExitStack
    with tile.TileContext(nc) as tc, ExitStack() as ctx:
        ep = ctx.enter_context
        consts = ep(tc.tile_pool(name="consts", bufs=1))
        wpool = ep(tc.tile_pool(name="wpool", bufs=1))
        xpool = ep(tc.tile_pool(name="xpool", bufs=1))
        rp = ep(tc.tile_pool(name="rp", bufs=1))
        stage = ep(tc.tile_pool(name="stage", bufs=2))
        small = ep(tc.tile_pool(name="small", bufs=1))
        psum = ep(tc.tile_pool(name="psum", bufs=1, space="PSUM"))
        if True:
            # ---- constants ----
            ids_t = consts.tile([128, NT], I32)
            nc.sync.dma_start(out=ids_t, in_=ids_d)
            onesr = consts.tile([1, 128], BF16)
            nc.sync.dma_start(out=onesr, in_=onesr_d)
            onesc = consts.tile([128, 1], BF16)
            nc.sync.dma_start(out=onesc, in_=onesc_d)
            ident = consts.tile([128, 128], F32)
            nc.sync.dma_start(out=ident, in_=ident_d)
            sel2a = consts.tile([1, 128], BF16)
            nc.sync.dma_start(out=sel2a, in_=sel2_d[0:1, :])
            sel2b = consts.tile([1, 128], BF16)
            nc.sync.dma_start(out=sel2b, in_=sel2_d[1:2, :])
            fln_t = consts.tile([128, 2], F32)
            nc.sync.dma_start(out=fln_t, in_=fln_d)
            eps_t = consts.tile([1, 1], F32)
            nc.vector.memset(eps_t, EPS)

            # ---- persistent activations ----
            xT = xpool.tile([128, 2, S], F32, tag="xT")
            qTt = xpool.tile([128, 2, S], BF16, tag="qT")
            kTt = xpool.tile([128, 2, S], BF16, tag="kT")
            oT = xpool.tile([128, 4, S], BF16, tag="oT")
            nc.vector.memset(oT, 0.0)
            VW = H * 33 + 32          # 296: 8x(32 v + 1 ones) + zero tail
            vA = xpool.tile([128, NT, VW], BF16, tag="vA")
            # ones columns + zero tail persist across layers (evacs only
            # write the 32 v columns of each head)
            vA4 = vA[:, :, 0:H * 33].rearrange("p a (h c) -> p a h c", c=33)
            nc.vector.memset(vA4[:, :, :, 32:33], 1.0)
            nc.vector.memset(vA[:, :, H * 33:VW], 0.0)
            # random-block gathers (per layer): kRand column-gathers kT for
            # the two rand blocks of each n (per hg); vRand partition-gathers
            # v_aug (hg-independent, once per layer).
            vRand = xpool.tile([128, NB, VW], BF16, tag="vRand")

            # ---- embedding gather + transpose to D-major ----
            for t in range(NT):
                xtok = stage.tile([128, D], F32, tag="xtok")
                nc.gpsimd.indirect_dma_start(
                    out=xtok, out_offset=None, in_=emb_d,
                    in_offset=bass.IndirectOffsetOnAxis(ap=ids_t[:, t:t + 1], axis=0),
                )
                for c in range(2):
                    tp = psum.tile([128, 512], F32, tag="b1", bufs=4)
                    nc.tensor.transpose(tp[:, 0:128],
                                        xtok[:, c * 128:(c + 1) * 128], ident)
                    nc.vector.tensor_copy(xT[:, c, t * 128:(t + 1) * 128],
                                          tp[:, 0:128])

            def rmsnorm_factors():
                """-> (rh, rl) bf16 hi/lo split of per-token rstd [1, S]."""
                sq = xpool.tile([128, 2, S], BF16, tag="xn")
                for po in range(2):
                    nc.vector.tensor_tensor(sq[:, po, :], xT[:, po, :],
                                            xT[:, po, :], op=ALU.mult)
                rstd = small.tile([1, S], F32, tag="rstd")
                for nt in range(4):
                    sl = slice(nt * 512, (nt + 1) * 512)
                    ssq = psum.tile([128, 512], F32, tag="b1", bufs=4)
                    for c in range(2):
                        nc.tensor.matmul(ssq[0:1, :], onesc, sq[:, c, sl],
                                         start=(c == 0), stop=(c == 1))
                    nc.scalar.activation(rstd[:, sl], ssq[0:1, :], AF.Ln,
                                         bias=eps_t[:, :], scale=1.0 / D)
                nc.scalar.activation(rstd, rstd, AF.Exp, scale=-0.5)
                rh = small.tile([1, S], BF16, tag="rh")
                nc.vector.tensor_copy(rh, rstd)
                rl = small.tile([1, S], BF16, tag="rl")
                nc.vector.tensor_tensor(rl, rstd, rh, op=ALU.subtract)
                return rh, rl

            def apply_rstd(dst, rh, rl, src_po, extra=None):
                """dst[:, po, sl] = xT[:, po, sl] * bcast(rstd) (chunked)."""
                for nt in range(4):
                    sl = slice(nt * 512, (nt + 1) * 512)
                    bc = psum.tile([128, 512], F32, tag="b1", bufs=4)
                    nc.tensor.matmul(bc, onesr, rh[:, sl],
                                     start=True, stop=False)
                    nc.tensor.matmul(bc, onesr, rl[:, sl],
                                     start=False, stop=True)
                    for po in range(2):
                        nc.vector.tensor_tensor(dst[:, po, sl], xT[:, po, sl],
                                                bc, op=ALU.mult)
                        if extra is not None:
                            nc.vector.tensor_scalar(
                                out=dst[:, po, sl], in0=dst[:, po, sl],
                                scalar1=extra[:, po:po + 1], scalar2=None,
                                op0=ALU.mult)

            def make_xn():
                rh, rl = rmsnorm_factors()
                xn = xpool.tile([128, 2, S], BF16, tag="xn")
                apply_rstd(xn, rh, rl, None)
                return xn

            n_layers = 0 if phase == "embed" else (1 if phase != "full" else L)
            for l in range(n_layers):
                # ---- layer weights ----
                wq_t = wpool.tile([128, 2, D], BF16, tag="wq")
                wk_t = wpool.tile([128, 2, D], BF16, tag="wk")
                wv_t = wpool.tile([128, 2, D], BF16, tag="wv")
                for c in range(2):
                    nc.sync.dma_start(out=wq_t[:, c, :], in_=wq_d[l, c])
                    nc.sync.dma_start(out=wk_t[:, c, :], in_=wk_d[l, c])
                    nc.sync.dma_start(out=wv_t[:, c, :], in_=wv_d[l, c])
                wop_t = wpool.tile([128, 4, D], BF16, tag="wop")
                for hp in range(4):
                    nc.sync.dma_start(out=wop_t[:, hp, :], in_=wop_d[l, hp])
                w1_t = wpool.tile([128, 2, FF], BF16, tag="w1")
                for c in range(2):
                    nc.sync.dma_start(out=w1_t[:, c, :], in_=w1_d[l, c])
                w2_t = wpool.tile([128, 8, D], BF16, tag="w2")
                for kc in range(8):
                    nc.sync.dma_start(out=w2_t[:, kc, :], in_=w2_d[l, kc])

                # ---- norm 1 + QKV ----
                xn = make_xn()
                for (wt, dstT) in ((wq_t, qTt), (wk_t, kTt)):
                    for po in range(2):
                        for nt in range(4):
                            sl = slice(nt * 512, (nt + 1) * 512)
                            pp = psum.tile([128, 512], F32, tag="b1", bufs=4)
                            for c in range(2):
                                nc.tensor.matmul(
                                    pp,
                                    wt[:, c, po * 128:(po + 1) * 128],
                                    xn[:, c, sl],
                                    start=(c == 0), stop=(c == 1))
                            nc.vector.tensor_copy(dstT[:, po, sl], pp)
                for sg in range(8):
                    vp = psum.tile([128, 512], F32, tag="b1", bufs=4)
                    for stl in range(2):
                        st = sg * 2 + stl
                        for c in range(2):
                            nc.tensor.matmul(
                                vp[:, stl * 256:(stl + 1) * 256],
                                xn[:, c, st * 128:(st + 1) * 128],
                                wv_t[:, c, :],
                                start=(c == 0), stop=(c == 1))
                    nc.vector.tensor_copy(
                        vA[:, sg * 2:(sg + 1) * 2, 0:H * 33]
                        .rearrange("p a (h c) -> p a h c", c=33)[:, :, :, 0:32],
                        vp.rearrange("p (a h c) -> p a h c", a=2, c=32))

                # ---- random-block v gathers (hg-independent) ----
                ksc = os.environ.get("K_SC", "wgrkv")
                if "v" in ksc:
                    for n in range(NB):
                        for s2 in range(2):
                            r = int(rand_idx[n, s2])
                            nc.gpsimd.dma_start(
                                out=vRand[s2 * 64:(s2 + 1) * 64, n, :],
                                in_=vA[(r % 2) * 64:(r % 2) * 64 + 64,
                                       r // 2, :])

                if phase == "qkv":
                    break
                sub = phase[4:] if phase.startswith("attn") else ""
                # ---- block-sparse attention ----
                for hg in range(2):
                    # per-hg k column gather for the rand blocks
                    kRand = xpool.tile([128, NB * 128], BF16, tag="kRand")
                    if "k" in ksc:
                        for n in range(NB):
                            for s2 in range(2):
                                r = int(rand_idx[n, s2])
                                nc.sync.dma_start(
                                    out=kRand[:, n * 128 + s2 * 64:
                                              n * 128 + (s2 + 1) * 64],
                                    in_=kTt[:, hg, r * 64:(r + 1) * 64])

                    # window-pair exp tiles: rolling 7-slot pool (pair p is
                    # dead once qt=(2p+2)//8's o-matmuls have read it).
                    # Score psum: [128, 4, 512] so each head quadrant (PE
                    # row position) owns a full psum bank — K=32 matmuls
                    # from different row quadrants into the same bank hang
                    # the PE. Two window pairs share one tile (col halves).
                    etw = {}
                    unit = [None]

                    def emit_pair(p, hg=hg, etw=etw, unit=unit):
                        q0 = max(2 * p - 1, 0)
                        q1 = min(2 * p + 2, NB - 1)
                        ncol = (q1 - q0 + 1) * 64   # 192 (edge pairs) or 256
                        if unit[0] is None:
                            unit[0] = psum.tile([128, 4, 512], F32, tag="sc",
                                                bufs=1, name="scu")
                            scw, half = unit[0], 0
                        else:
                            scw, half = unit[0], 256
                            unit[0] = None
                        for hh in range(4):
                            pb = 32 * hh
                            nc.tensor.matmul(
                                scw[0:128, hh, half:half + ncol],
                                kTt[pb:pb + 32, hg,
                                    2 * p * 64:(2 * p + 2) * 64],
                                qTt[pb:pb + 32, hg, q0 * 64:(q1 + 1) * 64],
                                tile_position=(pb, 0), start=True, stop=True)
                            if ncol < 256:
                                # fill unwritten cols: exp must read
                                # written psum
                                nc.tensor.matmul(
                                    scw[0:128, hh, half + ncol:half + 256],
                                    kTt[pb:pb + 32, hg, 0:128],
                                    qTt[pb:pb + 32, hg, 0:256 - ncol],
                                    tile_position=(pb, 0),
                                    start=True, stop=True)
                        et = xpool.tile([128, 4, 256], BF16, tag="etw",
                                        bufs=7)
                        nc.scalar.activation(et, scw[:, :, half:half + 256],
                                             AF.Exp)
                        if p == 0:
                            # n=0 window is [0,0,1]: kv block 0 counts twice
                            nc.vector.tensor_scalar(
                                out=et[0:64, 0:4, 0:64],
                                in0=et[0:64, 0:4, 0:64],
                                scalar1=2.0, scalar2=None, op0=ALU.mult)
                        if p == NP - 1:
                            # n=31 window is [30,31,31]: kv block 31 twice
                            nc.vector.tensor_scalar(
                                out=et[64:128, 0:4, 128:192],
                                in0=et[64:128, 0:4, 128:192],
                                scalar1=2.0, scalar2=None, op0=ALU.mult)
                        etw[p] = et

                    pnext = 0
                    # phase b: per 512-token q tile: global + rand scores,
                    # then o accumulation
                    for qt in range(4):
                        while pnext <= min(4 * qt + 4, NP - 1):
                            if "w" in ksc:
                                emit_pair(pnext)
                            pnext += 1
                        unit[0] = None   # don't span a unit across qt's
                        etg = stage.tile([128, 4, 512], BF16, tag="etg")
                        if "g" in ksc:
                            scg = psum.tile([128, 4, 512], F32, tag="sc",
                                            bufs=1)
                            for hh in range(4):
                                pb = 32 * hh
                                nc.tensor.matmul(
                                    scg[0:128, hh, :],
                                    kTt[pb:pb + 32, hg, 0:128],
                                    qTt[pb:pb + 32, hg,
                                        qt * 512:(qt + 1) * 512],
                                    tile_position=(pb, 0),
                                    start=True, stop=True)
                            nc.scalar.activation(etg, scg, AF.Exp)
                        etr = stage.tile([128, 4, 512], BF16, tag="etr")
                        if "r" in ksc:
                            scr = psum.tile([128, 4, 512], F32, tag="sc",
                                            bufs=1)
                            for ni in range(8):
                                n = qt * 8 + ni
                                for hh in range(4):
                                    pb = 32 * hh
                                    nc.tensor.matmul(
                                        scr[0:128, hh, ni * 64:(ni + 1) * 64],
                                        kRand[pb:pb + 32,
                                              n * 128:(n + 1) * 128],
                                        qTt[pb:pb + 32, hg,
                                            n * 64:(n + 1) * 64],
                                        tile_position=(pb, 0),
                                        start=True, stop=True)
                            nc.scalar.activation(etr, scr, AF.Exp)

                        if sub == "1":
                            continue
                        for ph in range(2):
                            op_ = psum.tile([128, 512], F32, tag="b1", bufs=4)
                            for hl in range(2):
                                h = hg * 4 + ph * 2 + hl
                                hh = ph * 2 + hl
                                hs = slice(h * 33, h * 33 + 64)
                                mms = []   # (out, lhsT, rhs, tpos)
                                # global (kv blocks 0,1), N=512
                                mms.append((
                                    op_[hl * 64:hl * 64 + 64, 0:512],
                                    vA[:, 0, hs], etg[:, hh, 0:512],
                                    (0, hl * 64)))
                                # window middles: q = 2p, 2p+1
                                for p in range(4 * qt, 4 * qt + 4):
                                    q0 = max(2 * p - 1, 0)
                                    c0 = (2 * p - q0) * 64
                                    co = ((2 * p) % 8) * 64
                                    mms.append((
                                        op_[hl * 64:hl * 64 + 64,
                                            co:co + 128],
                                        vA[:, p, hs],
                                        etw[p][:, hh, c0:c0 + 128],
                                        (0, hl * 64)))
                                # window lower edges: q = 2p-1 (kv 2p)
                                for p in range(4 * qt + 1,
                                               min(4 * qt + 5, NP)):
                                    q = 2 * p - 1
                                    co = (q % 8) * 64
                                    mms.append((
                                        op_[hl * 64:hl * 64 + 64, co:co + 64],
                                        vA[0:64, p, hs],
                                        etw[p][0:64, hh, 0:64],
                                        (0, hl * 64)))
                                # random (both blocks stacked via vRand)
                                for n in range(qt * 8, qt * 8 + 8):
                                    co = (n % 8) * 64
                                    mms.append((
                                        op_[hl * 64:hl * 64 + 64, co:co + 64],
                                        vRand[:, n, hs],
                                        etr[:, hh, co:co + 64],
                                        (0, hl * 64)))
                                # spacer, then upper edges (PE rows 64:128):
                                # row-disjoint matmuls on the same psum
                                # region must not overlap in the PE array.
                                uppers = []
                                for p in range(max(4 * qt - 1, 0),
                                               4 * qt + 3):
                                    q = 2 * p + 2
                                    if q > NB - 1 or q // 8 != qt:
                                        continue
                                    q0 = max(2 * p - 1, 0)
                                    c0 = (q - q0) * 64
                                    co = (q % 8) * 64
                                    uppers.append((
                                        op_[hl * 64:hl * 64 + 64, co:co + 64],
                                        vA[64:128, p, hs],
                                        etw[p][64:128, hh, c0:c0 + 64],
                                        (64, hl * 64)))
                                if uppers:
                                    mms.append((
                                        op_[hl * 64:hl * 64 + 32, 0:64],
                                        vA[:, 0, H * 33:H * 33 + 32],
                                        etg[:, hh, 0:64],
                                        (0, hl * 64)))
                                    mms.extend(uppers)
                                for i, (o_ap, l_ap, r_ap, tpos) in \
                                        enumerate(mms):
                                    nc.tensor.matmul(
                                        o_ap, l_ap, r_ap, tile_position=tpos,
                                        start=(i == 0),
                                        stop=(i == len(mms) - 1))
                            qsl = slice(qt * 512, (qt + 1) * 512)
                            nc.vector.tensor_copy(oT[:, hg * 2 + ph, qsl],
                                                  op_)

                if sub in ("1", "2"):
                    continue
                # ---- softmax normalization ----
                # Denominators sit (bf16) at oT rows 32/96. DMA-pack them to
                # [16, 8, 128], one exact reciprocal, DMA-unpack to row form.
                den16 = rp.tile([16, 8, 128], BF16, tag="den")
                for hp in range(4):
                    for r in range(2):
                        nc.gpsimd.dma_start(
                            out=den16[:, 2 * hp + r, :],
                            in_=oT[32 + 64 * r:33 + 64 * r, hp, :])
                den16R = rp.tile([16, 8, 128], BF16, tag="denR")
                with nc.allow_low_precision("softmax denom recip in bf16"):
                    nc.vector.reciprocal(den16R, den16)
                if sub == "3":
                    continue
                for hp in range(4):
                    R_hp = rp.tile([1, 2, S], BF16, tag="Rall", bufs=2)
                    for r in range(2):
                        nc.gpsimd.dma_start(
                            out=R_hp[:, r, :],
                            in_=den16R[:, 2 * hp + r, :])
                    for nt in range(4):
                        sl = slice(nt * 512, (nt + 1) * 512)
                        bc2 = psum.tile([128, 512], F32, tag="b1", bufs=4)
                        nc.tensor.matmul(bc2, sel2a, R_hp[:, 0, sl],
                                         start=True, stop=False)
                        nc.tensor.matmul(bc2, sel2b, R_hp[:, 1, sl],
                                         start=False, stop=True)
                        nc.vector.tensor_tensor(oT[:, hp, sl], oT[:, hp, sl],
                                                bc2, op=ALU.mult)

                if sub in ("3", "4"):
                    continue
                # ---- wo + residual ----
                for po in range(2):
                    for nt in range(4):
                        sl = slice(nt * 512, (nt + 1) * 512)
                        wp = psum.tile([128, 512], F32, tag="b1", bufs=4)
                        for hp in range(4):
                            nc.tensor.matmul(
                                wp, wop_t[:, hp, po * 128:(po + 1) * 128],
                                oT[:, hp, sl],
                                start=(hp == 0), stop=(hp == 3))
                        nc.vector.tensor_tensor(xT[:, po, sl], xT[:, po, sl],
                                                wp, op=ALU.add)

                if phase.startswith("attn"):
                    break
                # ---- norm 2 + FFN ----
                xn2 = make_xn()
                for nt in range(4):
                    sl = slice(nt * 512, (nt + 1) * 512)
                    fg = stage.tile([128, 8, 512], BF16, tag="f1g")
                    for po8 in range(8):
                        fp_ = psum.tile([128, 512], F32, tag="b1", bufs=4)
                        for c in range(2):
                            nc.tensor.matmul(
                                fp_, w1_t[:, c, po8 * 128:(po8 + 1) * 128],
                                xn2[:, c, sl],
                                start=(c == 0), stop=(c == 1))
                        nc.scalar.activation(fg[:, po8, :], fp_,
                                             AF.Gelu_apprx_tanh)
                    for po in range(2):
                        f2p = psum.tile([128, 512], F32, tag="b1", bufs=4)
                        for kc in range(8):
                            nc.tensor.matmul(
                                f2p, w2_t[:, kc, po * 128:(po + 1) * 128],
                                fg[:, kc, :],
                                start=(kc == 0), stop=(kc == 7))
                        nc.vector.tensor_tensor(xT[:, po, sl], xT[:, po, sl],
                                                f2p, op=ALU.add)

            # ---- final RMSNorm (with final_ln_w) + transpose out ----
            xnF = xpool.tile([128, 2, S], F32, tag="qT")
            if phase == "full":
                rh, rl = rmsnorm_factors()
                apply_rstd(xnF, rh, rl, None, extra=fln_t)
            else:
                for po in range(2):
                    nc.vector.tensor_copy(xnF[:, po, :], xT[:, po, :])
            for t in range(NT):
                osb = stage.tile([128, D], F32, tag="osb")
                for po in range(2):
                    tp = psum.tile([128, 512], F32, tag="b1", bufs=4)
                    nc.tensor.transpose(
                        tp[:, 0:128], xnF[:, po, t * 128:(t + 1) * 128], ident)
                    nc.vector.tensor_copy(osb[:, po * 128:(po + 1) * 128],
                                          tp[:, 0:128])
                nc.sync.dma_start(out=out_d[t * 128:(t + 1) * 128, :], in_=osb)

    nc.compile()
    return nc


def prep_in_maps(inputs):
    bf = ml_dtypes.bfloat16
    ids = np.asarray(inputs["input_ids"]).astype(np.int32)          # [8, S]
    rand_idx = np.asarray(inputs["rand_idx"]).astype(np.int32)      # [NB, 2]
    emb = np.ascontiguousarray(np.asarray(inputs["emb"], np.float32))
    ln1 = np.asarray(inputs["ln1_w"], np.float32)
    ln2 = np.asarray(inputs["ln2_w"], np.float32)
    wq = np.asarray(inputs["wq"], np.float32)
    wk = np.asarray(inputs["wk"], np.float32)
    wv = np.asarray(inputs["wv"], np.float32)
    wo = np.asarray(inputs["wo"], np.float32)
    w1 = np.asarray(inputs["w1"], np.float32)
    w2 = np.asarray(inputs["w2"], np.float32)
    fln = np.asarray(inputs["final_ln_w"], np.float32)

    scale = 1.0 / np.sqrt(DH)
    wq_p = np.ascontiguousarray(
        (wq * ln1[:, :, None] * scale).reshape(L, 2, 128, D)).astype(bf)
    wk_p = np.ascontiguousarray(
        (wk * ln1[:, :, None]).reshape(L, 2, 128, D)).astype(bf)
    wv_p = np.ascontiguousarray(
        (wv * ln1[:, :, None]).reshape(L, 2, 128, D)).astype(bf)
    wop = np.zeros((L, 4, 128, D), np.float32)
    for hp in range(4):
        wop[:, hp, 0:32, :] = wo[:, 64 * hp:64 * hp + 32, :]
        wop[:, hp, 64:96, :] = wo[:, 64 * hp + 32:64 * hp + 64, :]
    wop = wop.astype(bf)
    w1_p = np.ascontiguousarray(
        (w1 * ln2[:, :, None]).reshape(L, 2, 128, FF)).astype(bf)
    w2_p = np.ascontiguousarray(w2.reshape(L, 8, 128, D)).astype(bf)

    sel2 = np.zeros((2, 128), bf)
    sel2[0, :64] = 1.0
    sel2[1, 64:] = 1.0
    common = {
        "emb": emb,
        "wq": wq_p, "wk": wk_p, "wv": wv_p, "wop": wop,
        "w1": w1_p, "w2": w2_p,
        "onesr": np.ones((1, 128), bf),
        "onesc": np.ones((128, 1), bf),
        "ident": np.eye(128, dtype=np.float32),
        "sel2": sel2,
        "fln": np.ascontiguousarray(fln.reshape(2, 128).T),
    }
    in_maps = []
    for c in range(NCORES):
        m = dict(common)
        m["ids"] = np.ascontiguousarray(ids[c].reshape(NT, 128).T)
        in_maps.append(m)
    return in_maps, rand_idx


_NC_CACHE = {}


def get_nc(rand_idx):
    key = (os.environ.get("K_PHASE", "full"), os.environ.get("K_SC", "wgrkv"),
           rand_idx.tobytes())
    if key not in _NC_CACHE:
        _NC_CACHE[key] = build_kernel(rand_idx)
    return _NC_CACHE[key]


def kernel(**inputs):
    in_maps, rand_idx = prep_in_maps(inputs)
    nc = get_nc(rand_idx)
    res = bass_utils.run_bass_kernel_spmd(nc, in_maps, list(range(NCORES)),
                                          trace=False)
    out = np.stack([np.asarray(res.results[c]["out"], np.float32)
                    for c in range(NCORES)])
    return out
